# revision 9
# baseline (speedup 1.0000x reference)
"""Trainium2 Bass kernel for a 3-layer complex RBF network (v2).

Math per layer (complex y, G; real phi):
    dist_i = sum_j |y_j - G_ij|^2
    phi    = exp(-dist / (2 s))
    y_out  = W @ phi + b        (complex W, b)

Distribution (8 cores): shard the hidden axis I=4096 -> 512 rows of G / columns
of W per core.  dist/phi are computed fully locally per shard; the matvec
W[:, shard] @ phi_shard yields a full-length partial y that is AllReduce-summed
across cores (b/8 is added on every core's partial before the reduce).

v2 design notes (vs the v1 baseline at 617us):
  The problem is pure HBM streaming (76.8MB fp32 weights per core ~= 215us at
  358GB/s).  v1 lost 2.9x to (a) 36 xbar W-transposes whose 74k 256B packets
  round-robin-poisoned the same 16 SDMA engines that carry the weight stream,
  (b) issue-order stalls on the gpsimd ring, and (c) layer-boundary idles.

  v2 eliminates the PE matvec entirely -- and with it every transpose:
  - Weights stream natural-layout via SWDGE cast-DMA (fp32->bf16), in large
    slabs: W halves [128p, 16, 512] (32KB contiguous read per partition),
    G (r, c-pair) tiles [128p, 2, Op] (16KB runs).  The gpsimd queue carries
    ONLY this stream (plus the 3 AllReduce triggers at the end), so it never
    blocks on compute.
  - dist: DVE subtract (in place, bf16 2x) + ACT Square with accum_out.
  - phi = exp(max(dist * -1/(2s), -85)) with the clamp fused into the
    tensor_scalar combine (per-component clamp; exp(-170)=0 anyway).
  - W matvec = DVE tensor_tensor_reduce row-dots against phi broadcast
    [128, 512]: y[o] for o = h*2048 + 16p + k accumulates in fp32.
    phi [128i,4] reaches broadcast layout via one padded xbar transpose
    [128,128] + 256B-row flatten to DRAM + partition_broadcast back.
  - AllReduce payloads are bf16 for layers 1-2 (y is re-broadcast bf16 anyway)
    and fp32 for the final output layer.  b/8 is folded in on DVE, so no
    accum-DMA rides the gpsimd ring.
  Engines: PE/PSUM unused; DVE ~90us, ACT ~50us, all overlapped by the stream.

  Ring/pool deadlock audit (issue order is program order per engine; a
  dma_start's pool-slot WAR wait must never transitively need an AllReduce
  that sits LATER in the same engine queue):
    gpsimd: xcast, G1x4, W1x4, G2x4, W2x4, AR1, G3x4, W3x2, AR2, AR3.
      gpool bufs=6: G3 lands on G1's slots (consumed in dist1) and G2's
      slots (consumed in dist2, which only needs AR1 -- already triggered).
      wpool bufs=4: W2 reuses W1's slots (consumed by matvec1, pre-AR1);
      W3 reuses W2's (matvec2 needs only AR1).  No cycles.
"""

import numpy as np

P = 128
NCORES = 8
HID = 4096
IS = HID // NCORES          # 512: per-core shard of the hidden axis
NCH = IS // P               # 4 chunks of 128 (i = c*128 + p)
# (Oprev, Ol) for layers 1..3
DIMS = [(1024, 4096), (4096, 4096), (4096, 1024)]

_cache = {}


def _build_nc():
    import concourse.bacc as bacc
    import concourse.mybir as mybir
    import concourse.tile as tile

    f32 = mybir.dt.float32
    bf16 = mybir.dt.bfloat16
    AF = mybir.ActivationFunctionType
    ALU = mybir.AluOpType

    nc = bacc.Bacc(None)

    x = nc.dram_tensor("x", [2, 1024], f32, kind="ExternalInput")
    W, G, S, B = {}, {}, {}, {}
    for l, (Op, Ol) in enumerate(DIMS, start=1):
        W[l] = nc.dram_tensor(f"W{l}s", [2, Ol, IS], f32, kind="ExternalInput")
        G[l] = nc.dram_tensor(f"G{l}s", [2, IS, Op], f32, kind="ExternalInput")
        S[l] = nc.dram_tensor(f"s{l}s", [IS], f32, kind="ExternalInput")
        B[l] = nc.dram_tensor(f"b{l}f", [2, Ol], f32, kind="ExternalInput")
    out = nc.dram_tensor("out", [2, 1024], f32, kind="ExternalOutput")

    with tile.TileContext(nc) as tc:
        with (
            tc.tile_pool(name="gpool", bufs=6) as gpool,    # [128, 2, Op] bf16
            tc.tile_pool(name="wpool", bufs=4) as wpool,    # [128, 16, 512] bf16
            tc.tile_pool(name="ybc", bufs=2) as ybcp,       # [128, Op] bf16
            tc.tile_pool(name="prod", bufs=4) as prodp,     # [128, IS] bf16
            tc.tile_pool(name="small", bufs=1) as small,
            tc.tile_pool(name="dram", bufs=1, space="DRAM") as dramp,
        ):
            # ---------------- x -> bf16 -> broadcast, first on both rings -----
            # (layer-1 compute is gated on this; keep it ahead of the s/b
            # preloads in the scalar queue)
            xbf = dramp.tile([2, 1024], bf16, tag="xbf")
            nc.gpsimd.dma_start(xbf[:], x[:])   # DRAM->DRAM cast, t=0, no waits
            # input for the warm-up AllReduce (absorbs the ~40us first-use
            # cost of the ncfw collective path off the critical path)
            cw_in = dramp.tile([2, 16], f32, tag="cw_in")
            cw_out = dramp.tile([2, 16], f32, tag="cw_out")
            nc.scalar.dma_start(cw_in[:], x[:, 0:16])
            ybct = {}
            for r in range(2):
                yb = ybcp.tile([P, DIMS[0][0]], bf16, tag="ybc")
                nc.scalar.dma_start(yb[:], xbf[r : r + 1, :].partition_broadcast(P))
                ybct[(1, r)] = yb

            # ---------------- small preloads (scalar HWDGE ring) --------------
            n2s, btile = {}, {}
            for l, (Op, Ol) in enumerate(DIMS, start=1):
                s4 = small.tile([P, NCH], f32, tag=f"s4_{l}")
                nc.gpsimd.dma_start(s4[:], S[l][:].rearrange("(c p) -> p c", p=P))
                rec = small.tile([P, NCH], f32, tag=f"rec_{l}")
                nc.vector.reciprocal(rec[:], s4[:])
                t = small.tile([P, NCH], f32, tag=f"n2s_{l}")
                nc.vector.tensor_scalar_mul(t[:], rec[:], -0.5)
                n2s[l] = t

                # b/8 staged in the ysb column layout: o = h*(Ol/2) + p*K + k
                K = Ol // (2 * P)           # 16 for Ol=4096, 4 for Ol=1024
                bt = small.tile([P, 2 * 2 * K], f32, tag=f"bt_{l}")
                for r in range(2):
                    for h in range(2):
                        col = (r * 2 + h) * K
                        nc.scalar.dma_start(
                            bt[:, col : col + K],
                            B[l][r, h * P * K : (h + 1) * P * K].rearrange(
                                "(p k) -> p k", p=P
                            ),
                        )
                nc.vector.tensor_scalar_mul(bt[:], bt[:], 1.0 / NCORES)
                btile[l] = bt

            # ---------------- weight-stream emission (gpsimd SWDGE ring) ------
            gt = {}    # (l, r, cp) -> [128, 2, Op] bf16; i = (2cp+ci)*128 + p
            wt = {}    # (l, r, h)  -> [128, K, 512] bf16; o = h*(Ol/2) + p*K + k

            def emit_g_loads(l):
                Op = DIMS[l - 1][0]
                for r in range(2):
                    for cp in range(NCH // 2):
                        g = gpool.tile([P, 2, Op], bf16, tag="g")
                        nc.gpsimd.dma_start(
                            g[:],
                            G[l][r, cp * 2 * P : (cp + 1) * 2 * P, :].rearrange(
                                "(c p) j -> p c j", p=P
                            ),
                        )
                        gt[(l, r, cp)] = g

            def emit_w_loads(l):
                Ol = DIMS[l - 1][1]
                K = Ol // (2 * P)
                H = Ol // (P * K)           # 2 halves (1 for... always 2 here)
                for r in range(2):
                    for h in range(H):
                        w = wpool.tile([P, K, 512], bf16, tag="w")
                        nc.gpsimd.dma_start(
                            w[:],
                            W[l][r, h * P * K : (h + 1) * P * K, :].rearrange(
                                "(p k) i -> p k i", p=P
                            ),
                        )
                        wt[(l, r, h)] = w

            emit_g_loads(1)
            emit_w_loads(1)
            emit_g_loads(2)
            emit_w_loads(2)
            # G3/W3 + all AllReduce triggers are emitted inside the layer loop
            # below so their pool-slot WAR waits sit AFTER AR1 in program order.
            # Warm-up AllReduce: all weight descriptors are already emitted, so
            # blocking the gpsimd sequencer here costs nothing.
            nc.gpsimd.collective_compute(
                "AllReduce", ALU.add,
                replica_groups=[list(range(NCORES))],
                ins=[cw_in.opt()], outs=[cw_out.opt()],
            )

            # ---------------- per-layer compute --------------------------------
            junk2 = small.tile([P, 2], f32, tag="junk2")
            ccout_prev = None
            for l, (Op, Ol) in enumerate(DIMS, start=1):
                K = Ol // (2 * P)
                H = 2

                if l > 1:
                    # ccout is f32: broadcast into an f32 staging tile on the
                    # scalar HWDGE ring, cast to bf16 on DVE (keeps the gpsimd
                    # weight ring free of y plumbing)
                    for r in range(2):
                        ystg = small.tile([P, Op], f32, tag="ystg")
                        nc.scalar.dma_start(
                            ystg[:], ccout_prev[r : r + 1, :].partition_broadcast(P)
                        )
                        yb = ybcp.tile([P, Op], bf16, tag="ybc")
                        nc.vector.tensor_copy(yb[:], ystg[:])
                        ybct[(l, r)] = yb

                # ---- dist: DVE sub in place, ACT Square + accum ----
                dacc = small.tile([P, 2 * NCH], f32, tag=f"dacc_{l}")
                for r in range(2):
                    for cp in range(NCH // 2):
                        g = gt[(l, r, cp)]
                        for ci in range(2):
                            c = 2 * cp + ci
                            gs = g[:, ci, :]
                            nc.vector.tensor_sub(gs, gs, ybct[(l, r)][:])
                            nc.scalar.activation(
                                gs, gs, AF.Square,
                                accum_out=dacc[:, 2 * c + r : 2 * c + r + 1],
                            )

                # ---- phi = exp(clamped dist * -1/(2s)), then broadcast ----
                expin = small.tile([P, NCH], f32, tag=f"expin_{l}")
                phi = small.tile([P, NCH], bf16, tag=f"phi_{l}")
                for c in range(NCH):
                    # NOTE: the accumulator's reduce op follows op1 on HW, so
                    # op1 must stay `add`; clamp in a separate instruction.
                    nc.vector.tensor_scalar(
                        junk2[:], dacc[:, 2 * c : 2 * c + 2],
                        n2s[l][:, c : c + 1], 0.0, ALU.mult, ALU.add,
                        accum_out=expin[:, c : c + 1],
                    )
                    nc.vector.tensor_scalar_max(
                        expin[:, c : c + 1], expin[:, c : c + 1], -85.0
                    )
                nc.scalar.activation(phi[:], expin[:], AF.Exp)
                # scatter phi [128p, 4c] -> DRAM [i = c*128+p], then replicate
                # back as [128, 512] for the row-dot matvec
                phid = dramp.tile([1, IS], bf16, tag=f"phid_{l}")
                nc.scalar.dma_start(
                    phid[:].rearrange("o (c p) -> p (o c)", p=P), phi[:]
                )
                phib = small.tile([P, IS], bf16, tag=f"phib_{l}")
                nc.scalar.dma_start(phib[:], phid[0:1, :].partition_broadcast(P))

                # ---- y_partial[o] = sum_i W[o,i] phi_i ----
                # DVE mult per o-row; the free-axis reduce is split between
                # ACT (Copy + accum, ~0.7us/row) and DVE (tensor_scalar cache
                # reduce, ~0.3us/row) so the two engines work in parallel
                nact = max(1, (6 * K) // 16)
                ysb = small.tile([P, 2 * H * K], f32, tag=f"ysb_{l}")
                for r in range(2):
                    for h in range(H):
                        w = wt[(l, r, h)]
                        for k in range(K):
                            col = (r * H + h) * K + k
                            prod = prodp.tile([P, IS], bf16, tag="prod")
                            nc.vector.tensor_mul(prod[:], w[:, k, :], phib[:])
                            if k < nact:
                                nc.scalar.activation(
                                    prod[:], prod[:], AF.Copy,
                                    accum_out=ysb[:, col : col + 1],
                                )
                            else:
                                nc.vector.tensor_scalar(
                                    prod[:], prod[:], 1.0, 0.0,
                                    ALU.mult, ALU.add,
                                    accum_out=ysb[:, col : col + 1],
                                )

                # ---- + b/8, then AllReduce (f32) across the 8 cores ----
                ardt = f32
                ysbb = small.tile([P, 2 * H * K], ardt, tag=f"ysbb_{l}")
                nc.vector.tensor_add(ysbb[:], ysb[:], btile[l][:])
                ccin = dramp.tile([2, Ol], ardt, tag=f"ccin_{l}")
                ccout = dramp.tile([2, Ol], ardt, tag=f"ccout_{l}")
                for r in range(2):
                    for h in range(H):
                        col = (r * H + h) * K
                        nc.scalar.dma_start(
                            ccin[r, h * P * K : (h + 1) * P * K].rearrange(
                                "(p k) -> p k", p=P
                            ),
                            ysbb[:, col : col + K],
                        )
                nc.gpsimd.collective_compute(
                    "AllReduce",
                    ALU.add,
                    replica_groups=[list(range(NCORES))],
                    ins=[ccin.opt()],
                    outs=[ccout.opt()],
                )
                ccout_prev = ccout
                if l == 1:
                    # now safe: these WAR-wait on dist2/matvec2 slots, which
                    # only need AR1 -- whose trigger precedes them on this ring
                    emit_g_loads(3)
                    emit_w_loads(3)

            nc.gpsimd.dma_start(out[:], ccout_prev[:])

    nc.finalize()
    return nc


def _get_nc():
    if "nc" not in _cache:
        _cache["nc"] = _build_nc()
    return _cache["nc"]


def make_in_maps(inputs):
    """Host-side sharding: slice the hidden axis into 8 shards."""
    in_maps = []
    for c in range(NCORES):
        lo, hi = c * IS, (c + 1) * IS
        m = {"x": np.ascontiguousarray(inputs["x"], dtype=np.float32)}
        for l in range(1, 4):
            m[f"W{l}s"] = np.ascontiguousarray(inputs[f"W{l}"][:, :, lo:hi], dtype=np.float32)
            m[f"G{l}s"] = np.ascontiguousarray(inputs[f"G{l}"][:, lo:hi, :], dtype=np.float32)
            m[f"s{l}s"] = np.ascontiguousarray(inputs[f"s{l}"][lo:hi], dtype=np.float32)
            m[f"b{l}f"] = np.ascontiguousarray(inputs[f"b{l}"], dtype=np.float32)
        in_maps.append(m)
    return in_maps


def run(inputs, trace=False, **kw):
    from concourse.bass_utils import run_bass_kernel_spmd

    nc = _get_nc()
    in_maps = make_in_maps(inputs)
    res = run_bass_kernel_spmd(nc, in_maps, list(range(NCORES)), trace=trace, **kw)
    return res


def kernel(**inputs):
    res = run(inputs, trace=False)
    return np.asarray(res.results[0]["out"], dtype=np.float32)


# revision 10
# speedup vs baseline: 1.2085x; 1.2085x over previous
"""Trainium2 Bass kernel for a 3-layer complex RBF network (v2).

Math per layer (complex y, G; real phi):
    dist_i = sum_j |y_j - G_ij|^2
    phi    = exp(-dist / (2 s))
    y_out  = W @ phi + b        (complex W, b)

Distribution (8 cores): shard the hidden axis I=4096 -> 512 rows of G / columns
of W per core.  dist/phi are computed fully locally per shard; the matvec
W[:, shard] @ phi_shard yields a full-length partial y that is AllReduce-summed
across cores (b/8 is added on every core's partial before the reduce).

v2 design notes (vs the v1 baseline at 617us):
  The problem is pure HBM streaming (76.8MB fp32 weights per core ~= 215us at
  358GB/s).  v1 lost 2.9x to (a) 36 xbar W-transposes whose 74k 256B packets
  round-robin-poisoned the same 16 SDMA engines that carry the weight stream,
  (b) issue-order stalls on the gpsimd ring, and (c) layer-boundary idles.

  v2 eliminates the PE matvec entirely -- and with it every transpose:
  - Weights stream natural-layout via SWDGE cast-DMA (fp32->bf16), in large
    slabs: W halves [128p, 16, 512] (32KB contiguous read per partition),
    G (r, c-pair) tiles [128p, 2, Op] (16KB runs).  The gpsimd queue carries
    ONLY this stream (plus the 3 AllReduce triggers at the end), so it never
    blocks on compute.
  - dist: DVE subtract (in place, bf16 2x) + ACT Square with accum_out.
  - phi = exp(max(dist * -1/(2s), -85)) with the clamp fused into the
    tensor_scalar combine (per-component clamp; exp(-170)=0 anyway).
  - W matvec = DVE tensor_tensor_reduce row-dots against phi broadcast
    [128, 512]: y[o] for o = h*2048 + 16p + k accumulates in fp32.
    phi [128i,4] reaches broadcast layout via one padded xbar transpose
    [128,128] + 256B-row flatten to DRAM + partition_broadcast back.
  - AllReduce payloads are bf16 for layers 1-2 (y is re-broadcast bf16 anyway)
    and fp32 for the final output layer.  b/8 is folded in on DVE, so no
    accum-DMA rides the gpsimd ring.
  Engines: PE/PSUM unused; DVE ~90us, ACT ~50us, all overlapped by the stream.

  Ring/pool deadlock audit (issue order is program order per engine; a
  dma_start's pool-slot WAR wait must never transitively need an AllReduce
  that sits LATER in the same engine queue):
    gpsimd: xcast, G1x4, W1x4, G2x4, W2x4, AR1, G3x4, W3x2, AR2, AR3.
      gpool bufs=6: G3 lands on G1's slots (consumed in dist1) and G2's
      slots (consumed in dist2, which only needs AR1 -- already triggered).
      wpool bufs=4: W2 reuses W1's slots (consumed by matvec1, pre-AR1);
      W3 reuses W2's (matvec2 needs only AR1).  No cycles.
"""

import numpy as np

P = 128
NCORES = 8
HID = 4096
IS = HID // NCORES          # 512: per-core shard of the hidden axis
NCH = IS // P               # 4 chunks of 128 (i = c*128 + p)
# (Oprev, Ol) for layers 1..3
DIMS = [(1024, 4096), (4096, 4096), (4096, 1024)]

_cache = {}


def _build_nc():
    import concourse.bacc as bacc
    import concourse.mybir as mybir
    import concourse.tile as tile

    f32 = mybir.dt.float32
    bf16 = mybir.dt.bfloat16
    AF = mybir.ActivationFunctionType
    ALU = mybir.AluOpType

    nc = bacc.Bacc(None)

    x = nc.dram_tensor("x", [2, 1024], f32, kind="ExternalInput")
    W, G, S, B = {}, {}, {}, {}
    for l, (Op, Ol) in enumerate(DIMS, start=1):
        W[l] = nc.dram_tensor(f"W{l}s", [2, Ol, IS], f32, kind="ExternalInput")
        G[l] = nc.dram_tensor(f"G{l}s", [2, IS, Op], f32, kind="ExternalInput")
        S[l] = nc.dram_tensor(f"s{l}s", [IS], f32, kind="ExternalInput")
        B[l] = nc.dram_tensor(f"b{l}f", [2, Ol], f32, kind="ExternalInput")
    out = nc.dram_tensor("out", [2, 1024], f32, kind="ExternalOutput")

    with tile.TileContext(nc) as tc:
        with (
            tc.tile_pool(name="gpool", bufs=6) as gpool,    # [128, 2, Op] bf16
            tc.tile_pool(name="wpool", bufs=4) as wpool,    # [128, 16, 512] bf16
            tc.tile_pool(name="ybc", bufs=2) as ybcp,       # [128, Op] bf16
            tc.tile_pool(name="prod", bufs=4) as prodp,     # [128, IS] bf16
            tc.tile_pool(name="small", bufs=1) as small,
            tc.tile_pool(name="dram", bufs=1, space="DRAM") as dramp,
        ):
            # ---------------- x -> bf16 -> broadcast, first on both rings -----
            # (layer-1 compute is gated on this; keep it ahead of the s/b
            # preloads in the scalar queue)
            xbf = dramp.tile([2, 1024], bf16, tag="xbf")
            nc.gpsimd.dma_start(xbf[:], x[:])   # DRAM->DRAM cast, t=0, no waits
            # input for the warm-up AllReduce (absorbs the ~40us first-use
            # cost of the ncfw collective path off the critical path)
            cw_in = dramp.tile([2, 16], f32, tag="cw_in")
            cw_out = dramp.tile([2, 16], f32, tag="cw_out")
            nc.scalar.dma_start(cw_in[:], x[:, 0:16])
            ybct = {}
            for r in range(2):
                yb = ybcp.tile([P, DIMS[0][0]], bf16, tag="ybc")
                nc.scalar.dma_start(yb[:], xbf[r : r + 1, :].partition_broadcast(P))
                ybct[(1, r)] = yb

            # ---------------- small preloads (scalar HWDGE ring) --------------
            n2s, btile = {}, {}
            for l, (Op, Ol) in enumerate(DIMS, start=1):
                s4 = small.tile([P, NCH], f32, tag=f"s4_{l}")
                nc.gpsimd.dma_start(s4[:], S[l][:].rearrange("(c p) -> p c", p=P))
                rec = small.tile([P, NCH], f32, tag=f"rec_{l}")
                nc.vector.reciprocal(rec[:], s4[:])
                t = small.tile([P, NCH], f32, tag=f"n2s_{l}")
                nc.vector.tensor_scalar_mul(t[:], rec[:], -0.5)
                n2s[l] = t

                # b/8 staged in the ysb column layout: o = h*(Ol/2) + p*K + k
                K = Ol // (2 * P)           # 16 for Ol=4096, 4 for Ol=1024
                bt = small.tile([P, 2 * 2 * K], f32, tag=f"bt_{l}")
                for r in range(2):
                    for h in range(2):
                        col = (r * 2 + h) * K
                        nc.scalar.dma_start(
                            bt[:, col : col + K],
                            B[l][r, h * P * K : (h + 1) * P * K].rearrange(
                                "(p k) -> p k", p=P
                            ),
                        )
                nc.vector.tensor_scalar_mul(bt[:], bt[:], 1.0 / NCORES)
                btile[l] = bt

            # ---------------- weight-stream emission (gpsimd SWDGE ring) ------
            gt = {}    # (l, r, cp) -> [128, 2, Op] bf16; i = (2cp+ci)*128 + p
            wt = {}    # (l, r, h)  -> [128, K, 512] bf16; o = h*(Ol/2) + p*K + k

            def emit_g_loads(l):
                Op = DIMS[l - 1][0]
                for r in range(2):
                    for cp in range(NCH // 2):
                        g = gpool.tile([P, 2, Op], bf16, tag="g")
                        nc.gpsimd.dma_start(
                            g[:],
                            G[l][r, cp * 2 * P : (cp + 1) * 2 * P, :].rearrange(
                                "(c p) j -> p c j", p=P
                            ),
                        )
                        gt[(l, r, cp)] = g

            def emit_w_loads(l):
                Ol = DIMS[l - 1][1]
                K = Ol // (2 * P)
                H = Ol // (P * K)           # 2 halves (1 for... always 2 here)
                for r in range(2):
                    for h in range(H):
                        w = wpool.tile([P, K, 512], bf16, tag="w")
                        nc.gpsimd.dma_start(
                            w[:],
                            W[l][r, h * P * K : (h + 1) * P * K, :].rearrange(
                                "(p k) i -> p k i", p=P
                            ),
                        )
                        wt[(l, r, h)] = w

            emit_g_loads(1)
            emit_w_loads(1)
            # Warm-up AllReduce: absorbs the ~40us first-use cost of the ncfw
            # collective path while W1+G1 (20MB) are still draining.  It must
            # sit before any emission whose pool-slot WAR depends on layer-1
            # compute (W2/G3), else it fires late and queues ahead of AR1.
            nc.gpsimd.collective_compute(
                "AllReduce", ALU.add,
                replica_groups=[list(range(NCORES))],
                ins=[cw_in.opt()], outs=[cw_out.opt()],
            )
            emit_g_loads(2)
            emit_w_loads(2)
            # G3/W3 + all AllReduce triggers are emitted inside the layer loop
            # below so their pool-slot WAR waits sit AFTER AR1 in program order.

            # ---------------- per-layer compute --------------------------------
            junk2 = small.tile([P, 2], f32, tag="junk2")
            ccout_prev = None
            for l, (Op, Ol) in enumerate(DIMS, start=1):
                K = Ol // (2 * P)
                H = 2

                if l > 1:
                    # ccout is f32: broadcast into an f32 staging tile on the
                    # scalar HWDGE ring, cast to bf16 on DVE (keeps the gpsimd
                    # weight ring free of y plumbing)
                    for r in range(2):
                        ystg = small.tile([P, Op], f32, tag="ystg")
                        nc.scalar.dma_start(
                            ystg[:], ccout_prev[r : r + 1, :].partition_broadcast(P)
                        )
                        yb = ybcp.tile([P, Op], bf16, tag="ybc")
                        nc.vector.tensor_copy(yb[:], ystg[:])
                        ybct[(l, r)] = yb

                # ---- dist: DVE sub in place, ACT Square + accum ----
                dacc = small.tile([P, 2 * NCH], f32, tag=f"dacc_{l}")
                for r in range(2):
                    for cp in range(NCH // 2):
                        g = gt[(l, r, cp)]
                        for ci in range(2):
                            c = 2 * cp + ci
                            gs = g[:, ci, :]
                            nc.vector.tensor_sub(gs, gs, ybct[(l, r)][:])
                            nc.scalar.activation(
                                gs, gs, AF.Square,
                                accum_out=dacc[:, 2 * c + r : 2 * c + r + 1],
                            )

                # ---- phi = exp(clamped dist * -1/(2s)), then broadcast ----
                expin = small.tile([P, NCH], f32, tag=f"expin_{l}")
                for c in range(NCH):
                    # NOTE: the accumulator's reduce op follows op1 on HW, so
                    # op1 must stay `add`; clamp in a separate instruction.
                    nc.vector.tensor_scalar(
                        junk2[:], dacc[:, 2 * c : 2 * c + 2],
                        n2s[l][:, c : c + 1], 0.0, ALU.mult, ALU.add,
                        accum_out=expin[:, c : c + 1],
                    )
                    nc.vector.tensor_scalar_max(
                        expin[:, c : c + 1], expin[:, c : c + 1], -85.0
                    )
                # phi [128p, 4c] -> i-ordered row -> broadcast [128, 512].
                # A direct scatter DMA (512x 2B HBM writes) drains at RMW
                # speed (~45us) -- use the padded xbar transpose + contiguous
                # flatten instead.
                phiP = small.tile([P, P], bf16, tag=f"phiP_{l}")
                phiT = small.tile([P, P], bf16, tag=f"phiT_{l}")
                nc.vector.memset(phiP[:], 0.0)
                nc.scalar.activation(phiP[:, 0:NCH], expin[:], AF.Exp)
                nc.sync.dma_start(phiT[:], phiP[:], transpose=True)
                phid = dramp.tile([1, IS], bf16, tag=f"phid_{l}")
                nc.scalar.dma_start(
                    phid[:].rearrange("o (c p) -> (o c) p", c=NCH), phiT[0:NCH, :]
                )
                phib = small.tile([P, IS], bf16, tag=f"phib_{l}")
                nc.scalar.dma_start(phib[:], phid[0:1, :].partition_broadcast(P))

                # ---- y_partial[o] = sum_i W[o,i] phi_i ----
                # DVE mult per o-row; the free-axis reduce is split between
                # ACT (Copy + accum, ~0.7us/row) and DVE (tensor_scalar cache
                # reduce, ~0.3us/row) so the two engines work in parallel
                nact = max(1, (6 * K) // 16)
                ysb = small.tile([P, 2 * H * K], f32, tag=f"ysb_{l}")
                for r in range(2):
                    for h in range(H):
                        w = wt[(l, r, h)]
                        for k in range(K):
                            col = (r * H + h) * K + k
                            prod = prodp.tile([P, IS], bf16, tag="prod")
                            nc.vector.tensor_mul(prod[:], w[:, k, :], phib[:])
                            if k < nact:
                                nc.scalar.activation(
                                    prod[:], prod[:], AF.Copy,
                                    accum_out=ysb[:, col : col + 1],
                                )
                            else:
                                nc.vector.tensor_scalar(
                                    prod[:], prod[:], 1.0, 0.0,
                                    ALU.mult, ALU.add,
                                    accum_out=ysb[:, col : col + 1],
                                )

                # ---- + b/8, then AllReduce (f32) across the 8 cores ----
                ardt = f32
                ysbb = small.tile([P, 2 * H * K], ardt, tag=f"ysbb_{l}")
                nc.vector.tensor_add(ysbb[:], ysb[:], btile[l][:])
                ccin = dramp.tile([2, Ol], ardt, tag=f"ccin_{l}")
                ccout = dramp.tile([2, Ol], ardt, tag=f"ccout_{l}")
                for r in range(2):
                    for h in range(H):
                        col = (r * H + h) * K
                        nc.scalar.dma_start(
                            ccin[r, h * P * K : (h + 1) * P * K].rearrange(
                                "(p k) -> p k", p=P
                            ),
                            ysbb[:, col : col + K],
                        )
                nc.gpsimd.collective_compute(
                    "AllReduce",
                    ALU.add,
                    replica_groups=[list(range(NCORES))],
                    ins=[ccin.opt()],
                    outs=[ccout.opt()],
                )
                ccout_prev = ccout
                if l == 1:
                    # now safe: these WAR-wait on dist2/matvec2 slots, which
                    # only need AR1 -- whose trigger precedes them on this ring
                    emit_g_loads(3)
                    emit_w_loads(3)

            nc.gpsimd.dma_start(out[:], ccout_prev[:])

    nc.finalize()
    return nc


def _get_nc():
    if "nc" not in _cache:
        _cache["nc"] = _build_nc()
    return _cache["nc"]


def make_in_maps(inputs):
    """Host-side sharding: slice the hidden axis into 8 shards."""
    in_maps = []
    for c in range(NCORES):
        lo, hi = c * IS, (c + 1) * IS
        m = {"x": np.ascontiguousarray(inputs["x"], dtype=np.float32)}
        for l in range(1, 4):
            m[f"W{l}s"] = np.ascontiguousarray(inputs[f"W{l}"][:, :, lo:hi], dtype=np.float32)
            m[f"G{l}s"] = np.ascontiguousarray(inputs[f"G{l}"][:, lo:hi, :], dtype=np.float32)
            m[f"s{l}s"] = np.ascontiguousarray(inputs[f"s{l}"][lo:hi], dtype=np.float32)
            m[f"b{l}f"] = np.ascontiguousarray(inputs[f"b{l}"], dtype=np.float32)
        in_maps.append(m)
    return in_maps


def run(inputs, trace=False, **kw):
    from concourse.bass_utils import run_bass_kernel_spmd

    nc = _get_nc()
    in_maps = make_in_maps(inputs)
    res = run_bass_kernel_spmd(nc, in_maps, list(range(NCORES)), trace=trace, **kw)
    return res


def kernel(**inputs):
    res = run(inputs, trace=False)
    return np.asarray(res.results[0]["out"], dtype=np.float32)


# revision 13
# speedup vs baseline: 1.2102x; 1.0014x over previous
"""Trainium2 Bass kernel for a 3-layer complex RBF network (v2).

Math per layer (complex y, G; real phi):
    dist_i = sum_j |y_j - G_ij|^2
    phi    = exp(-dist / (2 s))
    y_out  = W @ phi + b        (complex W, b)

Distribution (8 cores): shard the hidden axis I=4096 -> 512 rows of G / columns
of W per core.  dist/phi are computed fully locally per shard; the matvec
W[:, shard] @ phi_shard yields a full-length partial y that is AllReduce-summed
across cores (b/8 is added on every core's partial before the reduce).

v2 design notes (vs the v1 baseline at 617us):
  The problem is pure HBM streaming (76.8MB fp32 weights per core ~= 215us at
  358GB/s).  v1 lost 2.9x to (a) 36 xbar W-transposes whose 74k 256B packets
  round-robin-poisoned the same 16 SDMA engines that carry the weight stream,
  (b) issue-order stalls on the gpsimd ring, and (c) layer-boundary idles.

  v2 eliminates the PE matvec entirely -- and with it every transpose:
  - Weights stream natural-layout via SWDGE cast-DMA (fp32->bf16), in large
    slabs: W halves [128p, 16, 512] (32KB contiguous read per partition),
    G (r, c-pair) tiles [128p, 2, Op] (16KB runs).  The gpsimd queue carries
    ONLY this stream (plus the 3 AllReduce triggers at the end), so it never
    blocks on compute.
  - dist: DVE subtract (in place, bf16 2x) + ACT Square with accum_out.
  - phi = exp(max(dist * -1/(2s), -85)) with the clamp fused into the
    tensor_scalar combine (per-component clamp; exp(-170)=0 anyway).
  - W matvec = DVE tensor_tensor_reduce row-dots against phi broadcast
    [128, 512]: y[o] for o = h*2048 + 16p + k accumulates in fp32.
    phi [128i,4] reaches broadcast layout via one padded xbar transpose
    [128,128] + 256B-row flatten to DRAM + partition_broadcast back.
  - AllReduce payloads are bf16 for layers 1-2 (y is re-broadcast bf16 anyway)
    and fp32 for the final output layer.  b/8 is folded in on DVE, so no
    accum-DMA rides the gpsimd ring.
  Engines: PE/PSUM unused; DVE ~90us, ACT ~50us, all overlapped by the stream.

  Ring/pool deadlock audit (issue order is program order per engine; a
  dma_start's pool-slot WAR wait must never transitively need an AllReduce
  that sits LATER in the same engine queue):
    gpsimd: xcast, G1x4, W1x4, G2x4, W2x4, AR1, G3x4, W3x2, AR2, AR3.
      gpool bufs=6: G3 lands on G1's slots (consumed in dist1) and G2's
      slots (consumed in dist2, which only needs AR1 -- already triggered).
      wpool bufs=4: W2 reuses W1's slots (consumed by matvec1, pre-AR1);
      W3 reuses W2's (matvec2 needs only AR1).  No cycles.
"""

import numpy as np

P = 128
NCORES = 8
HID = 4096
IS = HID // NCORES          # 512: per-core shard of the hidden axis
NCH = IS // P               # 4 chunks of 128 (i = c*128 + p)
# (Oprev, Ol) for layers 1..3
DIMS = [(1024, 4096), (4096, 4096), (4096, 1024)]

_cache = {}


def _build_nc():
    import concourse.bacc as bacc
    import concourse.mybir as mybir
    import concourse.tile as tile

    f32 = mybir.dt.float32
    bf16 = mybir.dt.bfloat16
    AF = mybir.ActivationFunctionType
    ALU = mybir.AluOpType

    nc = bacc.Bacc(None)

    x = nc.dram_tensor("x", [2, 1024], f32, kind="ExternalInput")
    W, G, S, B = {}, {}, {}, {}
    for l, (Op, Ol) in enumerate(DIMS, start=1):
        W[l] = nc.dram_tensor(f"W{l}s", [2, Ol, IS], f32, kind="ExternalInput")
        G[l] = nc.dram_tensor(f"G{l}s", [2, IS, Op], f32, kind="ExternalInput")
        S[l] = nc.dram_tensor(f"s{l}s", [IS], f32, kind="ExternalInput")
        B[l] = nc.dram_tensor(f"b{l}f", [2, Ol], f32, kind="ExternalInput")
    out = nc.dram_tensor("out", [2, 1024], f32, kind="ExternalOutput")

    with tile.TileContext(nc) as tc:
        with (
            tc.tile_pool(name="gpool", bufs=6) as gpool,    # [128, 2, Op] bf16
            tc.tile_pool(name="wpool", bufs=4) as wpool,    # [128, 16, 512] bf16
            tc.tile_pool(name="ybc", bufs=2) as ybcp,       # [128, Op] bf16
            tc.tile_pool(name="prod", bufs=4) as prodp,     # [128, IS] bf16
            tc.tile_pool(name="small", bufs=1) as small,
            tc.tile_pool(name="dram", bufs=1, space="DRAM") as dramp,
        ):
            # ---------------- x -> bf16 -> broadcast, first on both rings -----
            # (layer-1 compute is gated on this; keep it ahead of the s/b
            # preloads in the scalar queue)
            xbf = dramp.tile([2, 1024], bf16, tag="xbf")
            nc.gpsimd.dma_start(xbf[:], x[:])   # DRAM->DRAM cast, t=0, no waits
            ybct = {}
            for r in range(2):
                yb = ybcp.tile([P, DIMS[0][0]], bf16, tag="ybc")
                nc.scalar.dma_start(yb[:], xbf[r : r + 1, :].partition_broadcast(P))
                ybct[(1, r)] = yb

            # ---------------- small preloads (scalar HWDGE ring) --------------
            n2s, btile = {}, {}
            for l, (Op, Ol) in enumerate(DIMS, start=1):
                s4 = small.tile([P, NCH], f32, tag=f"s4_{l}")
                nc.gpsimd.dma_start(s4[:], S[l][:].rearrange("(c p) -> p c", p=P))
                rec = small.tile([P, NCH], f32, tag=f"rec_{l}")
                nc.vector.reciprocal(rec[:], s4[:])
                t = small.tile([P, NCH], f32, tag=f"n2s_{l}")
                nc.vector.tensor_scalar_mul(t[:], rec[:], -0.5)
                n2s[l] = t

                # b/8 staged in the ysb column layout: o = h*(Ol/2) + p*K + k
                K = Ol // (2 * P)           # 16 for Ol=4096, 4 for Ol=1024
                bt = small.tile([P, 2 * 2 * K], f32, tag=f"bt_{l}")
                for r in range(2):
                    for h in range(2):
                        col = (r * 2 + h) * K
                        nc.scalar.dma_start(
                            bt[:, col : col + K],
                            B[l][r, h * P * K : (h + 1) * P * K].rearrange(
                                "(p k) -> p k", p=P
                            ),
                        )
                nc.vector.tensor_scalar_mul(bt[:], bt[:], 1.0 / NCORES)
                btile[l] = bt

            # ---------------- weight-stream emission (gpsimd SWDGE ring) ------
            gt = {}    # (l, r, cp) -> [128, 2, Op] bf16; i = (2cp+ci)*128 + p
            wt = {}    # (l, r, h)  -> [128, K, 512] bf16; o = h*(Ol/2) + p*K + k

            def emit_g_loads(l):
                Op = DIMS[l - 1][0]
                for r in range(2):
                    for cp in range(NCH // 2):
                        g = gpool.tile([P, 2, Op], bf16, tag="g")
                        nc.gpsimd.dma_start(
                            g[:],
                            G[l][r, cp * 2 * P : (cp + 1) * 2 * P, :].rearrange(
                                "(c p) j -> p c j", p=P
                            ),
                        )
                        gt[(l, r, cp)] = g

            def emit_w_loads(l):
                Ol = DIMS[l - 1][1]
                K = Ol // (2 * P)
                H = Ol // (P * K)           # 2 halves (1 for... always 2 here)
                for r in range(2):
                    for h in range(H):
                        w = wpool.tile([P, K, 512], bf16, tag="w")
                        nc.gpsimd.dma_start(
                            w[:],
                            W[l][r, h * P * K : (h + 1) * P * K, :].rearrange(
                                "(p k) i -> p k i", p=P
                            ),
                        )
                        wt[(l, r, h)] = w

            emit_g_loads(1)
            emit_w_loads(1)
            emit_g_loads(2)
            emit_w_loads(2)
            # G3/W3 + all AllReduce triggers are emitted inside the layer loop
            # below so their pool-slot WAR waits sit AFTER AR1 in program order.

            # ---------------- per-layer compute --------------------------------
            junk2 = small.tile([P, 2], f32, tag="junk2")
            ccout_prev = None
            for l, (Op, Ol) in enumerate(DIMS, start=1):
                K = Ol // (2 * P)
                H = 2

                if l > 1:
                    # ccout is f32: broadcast into an f32 staging tile on the
                    # scalar HWDGE ring, cast to bf16 on DVE (keeps the gpsimd
                    # weight ring free of y plumbing)
                    for r in range(2):
                        ystg = small.tile([P, Op], f32, tag="ystg")
                        nc.scalar.dma_start(
                            ystg[:], ccout_prev[r : r + 1, :].partition_broadcast(P)
                        )
                        yb = ybcp.tile([P, Op], bf16, tag="ybc")
                        nc.vector.tensor_copy(yb[:], ystg[:])
                        ybct[(l, r)] = yb

                # ---- dist: DVE sub in place, ACT Square + accum ----
                dacc = small.tile([P, 2 * NCH], f32, tag=f"dacc_{l}")
                for r in range(2):
                    for cp in range(NCH // 2):
                        g = gt[(l, r, cp)]
                        for ci in range(2):
                            c = 2 * cp + ci
                            gs = g[:, ci, :]
                            nc.vector.tensor_sub(gs, gs, ybct[(l, r)][:])
                            nc.scalar.activation(
                                gs, gs, AF.Square,
                                accum_out=dacc[:, 2 * c + r : 2 * c + r + 1],
                            )

                # ---- phi = exp(clamped dist * -1/(2s)), then broadcast ----
                expin = small.tile([P, NCH], f32, tag=f"expin_{l}")
                for c in range(NCH):
                    # NOTE: the accumulator's reduce op follows op1 on HW, so
                    # op1 must stay `add`; clamp in a separate instruction.
                    nc.vector.tensor_scalar(
                        junk2[:], dacc[:, 2 * c : 2 * c + 2],
                        n2s[l][:, c : c + 1], 0.0, ALU.mult, ALU.add,
                        accum_out=expin[:, c : c + 1],
                    )
                    nc.vector.tensor_scalar_max(
                        expin[:, c : c + 1], expin[:, c : c + 1], -85.0
                    )
                # phi [128p, 4c] -> i-ordered row -> broadcast [128, 512].
                # A direct scatter DMA (512x 2B HBM writes) drains at RMW
                # speed (~45us) -- use the padded xbar transpose + contiguous
                # flatten instead.
                phiP = small.tile([P, P], bf16, tag=f"phiP_{l}")
                phiT = small.tile([P, P], bf16, tag=f"phiT_{l}")
                nc.vector.memset(phiP[:], 0.0)
                nc.scalar.activation(phiP[:, 0:NCH], expin[:], AF.Exp)
                nc.sync.dma_start(phiT[:], phiP[:], transpose=True)
                phid = dramp.tile([1, IS], bf16, tag=f"phid_{l}")
                nc.scalar.dma_start(
                    phid[:].rearrange("o (c p) -> (o c) p", c=NCH), phiT[0:NCH, :]
                )
                phib = small.tile([P, IS], bf16, tag=f"phib_{l}")
                nc.scalar.dma_start(phib[:], phid[0:1, :].partition_broadcast(P))

                # ---- y_partial[o] = sum_i W[o,i] phi_i ----
                # DVE mult per o-row; the free-axis reduce is split between
                # ACT (Copy + accum, ~0.7us/row) and DVE (tensor_scalar cache
                # reduce, ~0.3us/row) so the two engines work in parallel
                nact = max(1, (6 * K) // 16)
                ysb = small.tile([P, 2 * H * K], f32, tag=f"ysb_{l}")
                for r in range(2):
                    for h in range(H):
                        w = wt[(l, r, h)]
                        for k in range(K):
                            col = (r * H + h) * K + k
                            prod = prodp.tile([P, IS], bf16, tag="prod")
                            nc.vector.tensor_mul(prod[:], w[:, k, :], phib[:])
                            if k < nact:
                                nc.scalar.activation(
                                    prod[:], prod[:], AF.Copy,
                                    accum_out=ysb[:, col : col + 1],
                                )
                            else:
                                nc.vector.tensor_scalar(
                                    prod[:], prod[:], 1.0, 0.0,
                                    ALU.mult, ALU.add,
                                    accum_out=ysb[:, col : col + 1],
                                )

                # ---- + b/8, then AllReduce (f32) across the 8 cores ----
                ardt = f32
                ysbb = small.tile([P, 2 * H * K], ardt, tag=f"ysbb_{l}")
                nc.vector.tensor_add(ysbb[:], ysb[:], btile[l][:])
                ccin = dramp.tile([2, Ol], ardt, tag=f"ccin_{l}")
                ccout = dramp.tile([2, Ol], ardt, tag=f"ccout_{l}")
                for r in range(2):
                    for h in range(H):
                        col = (r * H + h) * K
                        nc.scalar.dma_start(
                            ccin[r, h * P * K : (h + 1) * P * K].rearrange(
                                "(p k) -> p k", p=P
                            ),
                            ysbb[:, col : col + K],
                        )
                nc.gpsimd.collective_compute(
                    "AllReduce",
                    ALU.add,
                    replica_groups=[list(range(NCORES))],
                    ins=[ccin.opt()],
                    outs=[ccout.opt()],
                )
                ccout_prev = ccout
                if l == 1:
                    # now safe: these WAR-wait on dist2/matvec2 slots, which
                    # only need AR1 -- whose trigger precedes them on this ring
                    emit_g_loads(3)
                    emit_w_loads(3)

            nc.gpsimd.dma_start(out[:], ccout_prev[:])

    nc.finalize()
    return nc


def _get_nc():
    if "nc" not in _cache:
        _cache["nc"] = _build_nc()
    return _cache["nc"]


def make_in_maps(inputs):
    """Host-side sharding: slice the hidden axis into 8 shards."""
    in_maps = []
    for c in range(NCORES):
        lo, hi = c * IS, (c + 1) * IS
        m = {"x": np.ascontiguousarray(inputs["x"], dtype=np.float32)}
        for l in range(1, 4):
            m[f"W{l}s"] = np.ascontiguousarray(inputs[f"W{l}"][:, :, lo:hi], dtype=np.float32)
            m[f"G{l}s"] = np.ascontiguousarray(inputs[f"G{l}"][:, lo:hi, :], dtype=np.float32)
            m[f"s{l}s"] = np.ascontiguousarray(inputs[f"s{l}"][lo:hi], dtype=np.float32)
            m[f"b{l}f"] = np.ascontiguousarray(inputs[f"b{l}"], dtype=np.float32)
        in_maps.append(m)
    return in_maps


def run(inputs, trace=False, **kw):
    from concourse.bass_utils import run_bass_kernel_spmd

    nc = _get_nc()
    in_maps = make_in_maps(inputs)
    res = run_bass_kernel_spmd(nc, in_maps, list(range(NCORES)), trace=trace, **kw)
    return res


def kernel(**inputs):
    res = run(inputs, trace=False)
    return np.asarray(res.results[0]["out"], dtype=np.float32)
